# revision 7
# baseline (speedup 1.0000x reference)
"""Trainium2 Bass kernel for nn_BLIPConceptPrefixModelV3 (topk_masking).

Math: reference's gather+softmax+mean collapses to per-token weights:
    h[b] = (1/C) * sum_s w[b,s] * qp[b,s,:],   w[b,s] = sum_c softmax16(qk[b,c,:])[s]
where softmax16 is softmax over the top-16 entries of each (b,c) row.
Top-16 selection is done in exp-space (all positive, so "remove" == "zero")
with the Max8 + MatchReplace DVE instructions: two max8 rounds give the
16th-largest value as a threshold; a fused scalar_tensor_tensor computes
E*(E>=t16) and its row-sum (softmax denominator) in one op.  The softmax
normalization (1/(C*denom)) enters as the moving operand of the
concept-reduction matmul, so it costs no extra element-wise pass.

Sharding: data-parallel over batch B=16 across 8 cores (2 batches/core),
weights replicated; no collectives.  Host marshals q into both natural
([s,d], for the h matmul) and d-major ([d,s], for the qk matmul) layouts.
"""

import os
import sys

sys.path.insert(0, "/opt/trn_rl_repo")

import numpy as np

B, S, D = 16, 577, 768
SP = S - 1  # 576 patch tokens
C, NCLS = 256, 1000
TOPK = 16
NCORES = 8
BPC = B // NCORES  # batches per core

# s-chunks for contractions over s (partition dim <= 128)
SCH = [(0, 128), (128, 128), (256, 128), (384, 128), (512, 64)]

last_exec_time_ns = None
_cached = {}


def _apply_tile_patch():
    """walrus CoreV3 codegen rejects >2 sync-waits on a CTRL (Drain)
    instruction; split the TileContext tail-drain's waits across a chain of
    single-wait SP drains."""
    from concourse.tile import TileContext
    import concourse.mybir as mybir

    if getattr(TileContext, "_drain_patched", False):
        return

    MAX_WAITS = 1

    def _split_excess_waits(nc):
        """walrus rejects instructions carrying more than a couple of
        sync-waits; move the excess onto preceding same-engine Drain
        carriers (engines execute their stream in block order, so the
        waits still complete before the original instruction issues)."""
        for f in nc.m.functions:
            for blk in f.blocks:
                insts = list(blk.instructions)
                out = []
                changed = False
                for ins in insts:
                    si = getattr(ins, "sync_info", None)
                    eng = getattr(ins, "engine", None)
                    if si is not None and eng is not None and len(si.on_wait) > MAX_WAITS:
                        waits = list(si.on_wait)
                        si.on_wait.clear()
                        si.on_wait.extend(waits[:MAX_WAITS])
                        extra = waits[MAX_WAITS:]
                        for i in range(0, len(extra), MAX_WAITS):
                            carrier = mybir.InstDrain(
                                name=f"{ins.name}-w{i}",
                                ins=[],
                                outs=[],
                                engine=eng,
                            )
                            carrier.sync_info = mybir.SyncInfo(
                                on_wait=list(extra[i : i + MAX_WAITS]), on_update=[]
                            )
                            nc.register_instruction(carrier, overwrite=True)
                            out.append(carrier)
                        changed = True
                    out.append(ins)
                if changed:
                    blk.instructions.clear()
                    blk.instructions.extend(out)

    def _patched(self, tick_clock, wait_clock):
        import concourse.tile as tile_mod

        drain_inst = self.nc.sync.drain()
        wait_clock.add_sem_waits(
            drain_inst.ins, tile_mod.ScopedClock({None: tick_clock.global_clock})
        )
        waits = list(drain_inst.ins.sync_info.on_wait)
        if len(waits) > 1:
            drain_inst.ins.sync_info.on_wait.clear()
            drain_inst.ins.sync_info.on_wait.append(waits[0])
            for sw in waits[1:]:
                d = self.nc.sync.drain()
                if d.ins.sync_info is None:
                    d.ins.sync_info = mybir.SyncInfo(on_wait=[], on_update=[])
                d.ins.sync_info.on_wait.append(sw)

        self.nc.all_engine_barrier()
        assert self.sems is not None
        popped = self.nc._tile_sem_poison_stack.pop()
        assert popped is self._sem_poison
        self.nc.clear_and_free_semaphores(list(self.sems.allocated().values()))
        self.nc.all_engine_barrier()

        _split_excess_waits(self.nc)

    TileContext._drain_and_barrier = _patched
    TileContext._drain_patched = True


def _build_nc():
    import concourse.bass as bass
    import concourse.mybir as mybir
    from concourse.tile import TileContext
    from concourse.masks import make_identity

    f32 = mybir.dt.float32
    Alu = mybir.AluOpType
    Act = mybir.ActivationFunctionType

    nc = bass.Bass()
    qT_d = nc.declare_dram_parameter("qT", [BPC, D, SP], f32, isOutput=False)
    qn_d = nc.declare_dram_parameter("qn", [BPC, SP, D], f32, isOutput=False)
    cwT_d = nc.declare_dram_parameter("cwT", [D, C], f32, isOutput=False)
    clswT_d = nc.declare_dram_parameter("clswT", [D, NCLS], f32, isOutput=False)
    clsb_d = nc.declare_dram_parameter("clsb", [1, NCLS], f32, isOutput=False)
    y_d = nc.declare_dram_parameter("y", [BPC, NCLS], f32, isOutput=True)

    ND = D // 128  # 6 d-chunks

    with TileContext(nc) as tc:
        with (
            tc.tile_pool(name="const", bufs=1) as constp,
            tc.tile_pool(name="qTp", bufs=1) as qTp,
            tc.tile_pool(name="qnp", bufs=1) as qnp,
            tc.tile_pool(name="emp", bufs=1) as emp,
            tc.tile_pool(name="ewp", bufs=3) as ewp,
            tc.tile_pool(name="smp", bufs=2) as smp,
        ):
            # ---- loads ----
            cw = []
            for d in range(ND):
                t = constp.tile([128, C], f32, tag=f"cw{d}")
                nc.sync.dma_start(out=t[:], in_=cwT_d[d * 128 : (d + 1) * 128, :])
                cw.append(t)
            qTt = {}
            for b in range(BPC):
                for d in range(ND):
                    t = qTp.tile([128, SP], f32, tag=f"qT{b}_{d}")
                    nc.sync.dma_start(
                        out=t[:], in_=qT_d[b, d * 128 : (d + 1) * 128, :]
                    )
                    qTt[b, d] = t
            qnt = {}
            for b in range(BPC):
                for sc, (s0, sz) in enumerate(SCH):
                    t = qnp.tile([128, D], f32, tag=f"qn{b}_{sc}")
                    nc.sync.dma_start(out=t[0:sz, :], in_=qn_d[b, s0 : s0 + sz, :])
                    qnt[b, sc] = t
            clst = []
            for d in range(ND):
                t = constp.tile([128, NCLS], f32, tag=f"cls{d}")
                nc.sync.dma_start(out=t[:], in_=clswT_d[d * 128 : (d + 1) * 128, :])
                clst.append(t)
            bias_t = constp.tile([1, NCLS], f32, tag="bias")
            nc.sync.dma_start(out=bias_t[:], in_=clsb_d[:])
            ones_t = constp.tile([1, 2], f32, tag="ones")
            nc.vector.memset(ones_t[:], 1.0)
            ident = constp.tile([128, 128], f32, tag="ident")
            make_identity(nc, ident[:])

            Em = {}
            R = {}
            hrow = [
                smp.tile([1, D], f32, tag=f"hrow{b}", bufs=1, name=f"hrow{b}")
                for b in range(BPC)
            ]

            with (
                tc.tile_pool(name="psqk", bufs=2, space="PSUM") as qkp,
                tc.tile_pool(name="psw", bufs=2, space="PSUM") as pswp,
                tc.tile_pool(name="psh", bufs=1, space="PSUM") as pshp,
            ):
                # ---- phase 1+2: qk matmul, exp, top-16 mask ----
                for b in range(BPC):
                    for ct in range(2):
                        p0 = qkp.tile([128, 288], f32, tag="p0")
                        p1 = qkp.tile([128, 288], f32, tag="p1")
                        for d in range(ND):
                            nc.tensor.matmul(
                                p0[:],
                                lhsT=cw[d][:, ct * 128 : (ct + 1) * 128],
                                rhs=qTt[b, d][:, 0:288],
                                start=(d == 0),
                                stop=(d == ND - 1),
                            )
                        for d in range(ND):
                            nc.tensor.matmul(
                                p1[:],
                                lhsT=cw[d][:, ct * 128 : (ct + 1) * 128],
                                rhs=qTt[b, d][:, 288:576],
                                start=(d == 0),
                                stop=(d == ND - 1),
                            )
                        E = ewp.tile([128, SP], f32, tag="E")
                        nc.scalar.activation(E[:, 0:288], p0[:], Act.Exp)
                        nc.scalar.activation(E[:, 288:576], p1[:], Act.Exp)
                        m8a = smp.tile([128, 8], f32, tag="m8a")
                        nc.vector.max(out=m8a[:], in_=E[:])
                        work = ewp.tile([128, SP], f32, tag="W")
                        nc.vector.match_replace(
                            out=work[:], in_to_replace=m8a[:], in_values=E[:],
                            imm_value=0.0,
                        )
                        m8b = smp.tile([128, 8], f32, tag="m8b")
                        nc.vector.max(out=m8b[:], in_=work[:])
                        em = emp.tile([128, SP], f32, tag=f"em{b}_{ct}")
                        den = smp.tile([128, 1], f32, tag="den")
                        nc.vector.scalar_tensor_tensor(
                            out=em[:], in0=E[:], scalar=m8b[:, 7:8], in1=E[:],
                            op0=Alu.is_ge, op1=Alu.mult, accum_out=den[:],
                        )
                        denC = smp.tile([128, 1], f32, tag="denC")
                        nc.vector.tensor_scalar_mul(denC[:], den[:], float(C))
                        r = smp.tile([128, 1], f32, tag=f"r{b}_{ct}", bufs=1)
                        nc.vector.reciprocal(r[:], denC[:])
                        Em[b, ct] = em
                        R[b, ct] = r

                # ---- phase 3: concept reduction (w) and h matmul ----
                for b in range(BPC):
                    pw = pswp.tile([128, 8], f32, tag="pw")
                    for sc, (s0, sz) in enumerate(SCH):
                        for ct in range(2):
                            nc.tensor.matmul(
                                pw[0:sz, sc : sc + 1],
                                lhsT=Em[b, ct][:, s0 : s0 + sz],
                                rhs=R[b, ct][:],
                                start=(ct == 0),
                                stop=(ct == 1),
                            )
                    wcol = smp.tile([128, 5], f32, tag="wcol")
                    nc.scalar.activation(wcol[:], pw[:, 0:5], Act.Copy)
                    ph0 = pshp.tile([1, 384], f32, tag="ph0")
                    ph1 = pshp.tile([1, 384], f32, tag="ph1")
                    for nh, ph in enumerate((ph0, ph1)):
                        for sc, (s0, sz) in enumerate(SCH):
                            nc.tensor.matmul(
                                ph[:],
                                lhsT=wcol[0:sz, sc : sc + 1],
                                rhs=qnt[b, sc][0:sz, nh * 384 : (nh + 1) * 384],
                                start=(sc == 0),
                                stop=(sc == len(SCH) - 1),
                            )
                        nc.scalar.activation(
                            hrow[b][0:1, nh * 384 : (nh + 1) * 384], ph[:], Act.Copy
                        )

            # ---- phase 4: transpose h, relu, classifier ----
            hT = smp.tile([128, ND, BPC], f32, tag="hT", bufs=1)
            ysb = smp.tile([BPC, NCLS], f32, tag="ysb", bufs=1)
            with (
                tc.tile_pool(name="pst", bufs=2, space="PSUM") as pstp,
                tc.tile_pool(name="psy", bufs=2, space="PSUM") as psyp,
            ):
                for d in range(ND):
                    for b in range(BPC):
                        pt = pstp.tile([128, 1], f32, tag="pt")
                        nc.tensor.transpose(
                            out=pt[:],
                            in_=hrow[b][0:1, d * 128 : (d + 1) * 128],
                            identity=ident[0:1, 0:1],
                        )
                        nc.scalar.activation(hT[:, d, b : b + 1], pt[:], Act.Relu)
                for nn in range(2):
                    py = psyp.tile([BPC, 500], f32, tag="py")
                    for d in range(ND):
                        nc.tensor.matmul(
                            py[:],
                            lhsT=hT[:, d, :],
                            rhs=clst[d][:, nn * 500 : (nn + 1) * 500],
                            start=(d == 0),
                            stop=False,
                        )
                    nc.tensor.matmul(
                        py[:],
                        lhsT=ones_t[0:1, 0:BPC],
                        rhs=bias_t[0:1, nn * 500 : (nn + 1) * 500],
                        start=False,
                        stop=True,
                    )
                    nc.scalar.activation(
                        ysb[:, nn * 500 : (nn + 1) * 500], py[:], Act.Copy
                    )
                    nc.sync.dma_start(
                        out=y_d[:, nn * 500 : (nn + 1) * 500],
                        in_=ysb[:, nn * 500 : (nn + 1) * 500],
                    )
    return nc


def _register_ntff_hook():
    """The staged antenv package lacks axon_hooks; synthesize it and register
    the ctypes NTFF profile hook so trace=True yields exec_time_ns."""
    import types

    if "antenv.axon_hooks" in sys.modules:
        return
    try:
        import antenv
        from trn_agent_boot.trn_boot import _ntff_profile_via_ctypes

        mod = types.ModuleType("antenv.axon_hooks")
        _hook = [None]
        mod.set_axon_ntff_profile_hook = lambda h: _hook.__setitem__(0, h)
        mod.get_axon_ntff_profile_hook = lambda: _hook[0]
        sys.modules["antenv.axon_hooks"] = mod
        antenv.axon_hooks = mod
        mod.set_axon_ntff_profile_hook(
            _ntff_profile_via_ctypes("/opt/axon/libaxon_pjrt.so")
        )
    except Exception as e:  # profiling is best-effort
        print(f"ntff hook registration failed: {e}", file=sys.stderr)


def kernel(q, concept_w, cls_w, cls_b, topk):
    global last_exec_time_ns
    assert int(topk) == TOPK, f"kernel hardcodes top-k=16, got {topk}"

    _apply_tile_patch()
    if os.environ.get("BLIP_TRACE"):
        _register_ntff_hook()
    from concourse.bass_utils import run_bass_kernel_spmd

    if "nc" not in _cached:
        _cached["nc"] = _build_nc()
    nc = _cached["nc"]

    q = np.asarray(q, dtype=np.float32)
    qp = np.ascontiguousarray(q[:, 1:, :])  # [B, 576, 768]
    qT = np.ascontiguousarray(qp.transpose(0, 2, 1))  # [B, 768, 576]
    cwT = np.ascontiguousarray(np.asarray(concept_w, dtype=np.float32).T)
    clswT = np.ascontiguousarray(np.asarray(cls_w, dtype=np.float32).T)
    clsb = np.ascontiguousarray(
        np.asarray(cls_b, dtype=np.float32).reshape(1, NCLS)
    )

    in_maps = []
    for core in range(NCORES):
        b0 = core * BPC
        in_maps.append(
            {
                "qT": np.ascontiguousarray(qT[b0 : b0 + BPC]),
                "qn": np.ascontiguousarray(qp[b0 : b0 + BPC]),
                "cwT": cwT,
                "clswT": clswT,
                "clsb": clsb,
            }
        )

    trace = bool(os.environ.get("BLIP_TRACE"))
    res = run_bass_kernel_spmd(nc, in_maps, list(range(NCORES)), trace=trace)
    last_exec_time_ns = res.exec_time_ns

    y = np.concatenate([res.results[i]["y"] for i in range(NCORES)], axis=0)
    return np.ascontiguousarray(y, dtype=np.float32)
